# revision 16
# baseline (speedup 1.0000x reference)
"""2-layer LSTM (B=128, T=256, F=H=1024) on 8 Trainium2 NeuronCores.

Pair-pipeline: 4 pairs of cores; pair g owns batch slice [32g, 32g+32).
The even core of each pair runs layer 0, the odd core layer 1 — the SAME
SPMD program, role encoded purely in per-core input data (weights, mask,
bias-kill sequence). Each layer's recurrence is core-local; h0 streams
even->odd through one pairwise AllGather per step, consumed with a lag of
LAG steps, so no collective sits on the serial critical path.

Per core, per step t the PE queue is ordered to hide the ACT/DVE cell
chain of step t-1 under work that does not depend on it:

    [bias mm(t) + in-part mm(t)]  [transpose h(t-1)]  [rec-part mm(t)]

gates(t) psum accumulates bias (ones-row matmul) + sel(t) @ W_in first
(sel = x for even cores, gathered h0 for odd, chosen by one
copy_predicated), and closes with hT(t-1) @ W_rec. The cell (sigmoid/tanh
+ DVE state update) runs on ACT/DVE while the PE chews the next step's
in-part, so the step period ~= PE busy time (~29.5 us).
"""

import numpy as np
import ml_dtypes

import concourse.bass as bass
import concourse.tile as tile
from concourse import mybir
from concourse.bass_utils import run_bass_kernel_spmd

SKIP_CC = SKIP_MM = SKIP_CELL = False
N_CORES = 8
N_PAIRS = 4
BS = 32            # batch rows per pair
B = 128
T_FULL = 256
F = 1024
H = 1024
G4 = 4 * H         # 4096 gate columns
KC = 8             # 128-row K chunks for a 1024 contraction
LAG = 4            # AllGather consume lag (steps)
PF = 3             # DMA prefetch depth (steps)
BF16 = mybir.dt.bfloat16
F32 = mybir.dt.float32
AF = mybir.ActivationFunctionType
SLAB = KC * BS     # 256: hT / x slab free width


def _patch_tail_drain():
    """walrus on this image only allows ONE sem-wait on CTRL-type (Drain/NoOp)
    instructions; Tile's kernel-tail drain accumulates one wait per pending
    queue/collective sem and trips that limit. Spread the waits over a chain
    of single-wait nops instead."""
    if getattr(tile.TileContext, "_tail_drain_patched", False):
        return
    from concourse.tile import ScopedClock

    def _drain_and_barrier(self, tick_clock, wait_clock):
        nc = self.nc
        probe = nc.sync.nop(nofuse=True, hint="tail_wait_probe")
        wait_clock.add_sem_waits(probe.ins, ScopedClock({None: tick_clock.global_clock}))
        si = probe.ins.sync_info
        waits = list(si.on_wait) if si is not None else []
        if len(waits) > 1:
            si.on_wait = waits[:1]
            for w in waits[1:]:
                n2 = nc.sync.nop(nofuse=True, hint="tail_wait_extra")
                n2.ins.sync_info = mybir.SyncInfo(on_wait=[w], on_update=[])
        nc.sync.drain()
        nc.all_engine_barrier()
        popped = nc._tile_sem_poison_stack.pop()
        assert popped is self._sem_poison
        nc.clear_and_free_semaphores(list(self.sems.allocated().values()))
        nc.all_engine_barrier()

    tile.TileContext._drain_and_barrier = _drain_and_barrier
    tile.TileContext._tail_drain_patched = True


def _hoist_multi_waits(nc: bass.Bass):
    """walrus on this image rejects >1 sem-wait per instruction. Hoist extra
    waits onto single-wait NoOps inserted just before the instruction on the
    same engine (engine FIFO order makes this equivalent)."""
    for blk in nc.main_func.blocks:
        idx = 0
        while idx < len(blk.instructions):
            inst = blk.instructions[idx]
            si = getattr(inst, "sync_info", None)
            if si is not None and len(si.on_wait) > 1:
                waits = list(si.on_wait)
                si.on_wait = waits[-1:]
                for w in waits[:-1]:
                    nop = mybir.InstNoOp(
                        name=nc.get_next_instruction_name(), ins=[], outs=[]
                    )
                    nop.engine = inst.engine
                    nop.sync_info = mybir.SyncInfo(on_wait=[w], on_update=[])
                    nc.register_instruction(nop)
                    blk.instructions.insert(idx, nop)
                    idx += 1
            idx += 1


def _ap_sig(arg):
    """Stable signature for a lowered AP argument (stationary operand)."""
    try:
        ml = arg.memory_location()
        name = ml.name if ml is not None else None
    except Exception:
        name = getattr(arg, "name", None)
    return (name, getattr(arg, "offset", None), str(getattr(arg, "ap", None)),
            str(getattr(arg, "dtype", None)))


def _dedup_ldweights(nc: bass.Bass):
    """Drop InstLdweights that reload the stationary already in the PE array.

    The PE keeps the loaded stationary across matmuls; only another
    InstLdweights or a self-loading matmul (transpose) changes it. Deleted
    instructions' sem waits/updates are merged onto the next PE instruction
    (engine FIFO order preserves timing semantics); _hoist_multi_waits runs
    afterwards to restore the single-wait invariant.
    """
    for blk in nc.main_func.blocks:
        cur_sig = None
        new_instructions = []
        pending_sync = []
        for inst in blk.instructions:
            eng = getattr(inst, "engine", None)
            if eng == mybir.EngineType.PE:
                if isinstance(inst, mybir.InstLdweights):
                    sig = _ap_sig(inst.ins[0])
                    if sig == cur_sig:
                        si = getattr(inst, "sync_info", None)
                        if si is not None and (si.on_wait or si.on_update):
                            pending_sync.append(si)
                        continue        # drop redundant reload
                    cur_sig = sig
                elif isinstance(inst, mybir.InstMatmult):
                    if getattr(inst, "is_transpose", False):
                        cur_sig = None  # transpose self-loads the identity
                if pending_sync:
                    si = getattr(inst, "sync_info", None)
                    if si is None:
                        si = mybir.SyncInfo(on_wait=[], on_update=[])
                        inst.sync_info = si
                    for ps in pending_sync:
                        si.on_wait = list(si.on_wait) + list(ps.on_wait)
                        si.on_update = list(si.on_update) + list(ps.on_update)
                    pending_sync = []
            new_instructions.append(inst)
        assert not pending_sync, "dangling sync from trailing Ldweights"
        blk.instructions[:] = new_instructions


def build_pair_nc(t_steps: int) -> bass.Bass:
    _patch_tail_drain()
    nc = bass.Bass()
    S = t_steps + LAG + 1

    xT = nc.dram_tensor("xT", [S, 128, SLAB], BF16, kind="ExternalInput")
    w_in = nc.dram_tensor("w_in", [128, KC * G4], BF16, kind="ExternalInput")
    w_rec = nc.dram_tensor("w_rec", [128, KC * G4], BF16, kind="ExternalInput")
    # batch-replicated bias rows; _kill is the variant used for t <= LAG
    # (zeros on odd cores so layer-1 state stays exactly 0 during pipe fill)
    bias_main = nc.dram_tensor("bias_main", [BS, G4], BF16,
                               kind="ExternalInput")
    bias_kill = nc.dram_tensor("bias_kill", [BS, G4], BF16,
                               kind="ExternalInput")
    masks = nc.dram_tensor("masks", [128, SLAB], mybir.dt.uint8,
                           kind="ExternalInput")
    ident = nc.dram_tensor("ident", [BS, BS], F32, kind="ExternalInput")
    y = nc.dram_tensor("y", [t_steps, 128, SLAB], BF16, kind="ExternalOutput")

    # pairwise replica groups: pair g = cores (2g, 2g+1)
    rg = [[2 * g, 2 * g + 1] for g in range(N_PAIRS)]

    with tile.TileContext(nc) as tc:
        with (
            tc.tile_pool(name="const", bufs=1) as cpool,
            tc.tile_pool(name="xbuf", bufs=PF + 2) as xpool,
            tc.tile_pool(name="agbuf", bufs=PF + 2) as agpool,
            tc.tile_pool(name="selbuf", bufs=3) as selpool,
            tc.tile_pool(name="hslab", bufs=3) as hpool,
            tc.tile_pool(name="state", bufs=1) as spool,
            tc.tile_pool(name="act", bufs=1) as apool,
            tc.tile_pool(name="pg", bufs=3, space="PSUM") as pg,
            tc.tile_pool(name="ptr", bufs=2, space="PSUM") as ptr,
            tc.tile_pool(name="dram", bufs=LAG + 4, space="DRAM") as dpool,
        ):
            # ---- constants resident in SBUF (weights loaded per K-chunk so
            # the first matmuls start after 1/8 of the load)
            w_in_sb = cpool.tile([128, KC * G4], BF16)
            w_rec_sb = cpool.tile([128, KC * G4], BF16)
            for k in range(KC):
                nc.sync.dma_start(w_in_sb[:, k * G4:(k + 1) * G4],
                                  w_in[:, k * G4:(k + 1) * G4])
            for k in range(KC):
                nc.sync.dma_start(w_rec_sb[:, k * G4:(k + 1) * G4],
                                  w_rec[:, k * G4:(k + 1) * G4])
            bias_main_sb = cpool.tile([BS, G4], BF16)
            bias_kill_sb = cpool.tile([BS, G4], BF16)
            masks_sb = cpool.tile([128, SLAB], mybir.dt.uint8)
            id_sb = cpool.tile([BS, BS], F32)
            nc.sync.dma_start(bias_main_sb[:], bias_main[:])
            nc.sync.dma_start(bias_kill_sb[:], bias_kill[:])
            nc.sync.dma_start(masks_sb[:], masks[:])
            nc.sync.dma_start(id_sb[:], ident[:])

            # ---- persistent state
            c_st = spool.tile([BS, H], F32)
            nc.vector.memset(c_st[:], 0.0)
            hT_init = cpool.tile([128, SLAB], BF16)
            nc.vector.memset(hT_init[:], 0.0)

            # ---- prefetched x / ag slabs
            x_slabs, ag_reads = {}, {}
            for t in range(min(PF, S)):
                xs = xpool.tile([128, SLAB], BF16, tag="x", name=f"xpre{t}")
                nc.sync.dma_start(xs[:], xT[t])
                x_slabs[t] = xs

            cc_outs = {}    # production step -> shared DRAM tile

            def build_sel(t):
                """sel(t) = x(t), overwritten with gathered h0T where mask=1."""
                sel = selpool.tile([128, SLAB], BF16, tag="sel", name=f"sel{t}")
                nc.vector.tensor_copy(sel[:], x_slabs.pop(t)[:])
                if t in ag_reads:
                    nc.vector.copy_predicated(sel[:], masks_sb[:],
                                              ag_reads.pop(t)[:])
                return sel

            sel_cur = build_sel(0)
            sel_next = None
            hT_prev = hT_init
            GATE_ORDER = (0, 2, 1, 3)  # i, g, f, o (W col blocks: i|f|g|o)

            for t in range(S):
                # ---------- prefetch DMAs for step t+PF
                tp = t + PF
                if tp < S:
                    xs = xpool.tile([128, SLAB], BF16, tag="x", name=f"x{tp}")
                    nc.sync.dma_start(xs[:], xT[tp])
                    x_slabs[tp] = xs
                    src = tp - LAG - 1
                    if src in cc_outs:
                        ag = agpool.tile([128, SLAB], BF16, tag="ag",
                                         name=f"ag{tp}")
                        nc.sync.dma_start(ag[:], cc_outs.pop(src)[0])
                        ag_reads[tp] = ag

                # ---------- gates(t): in-part (psum opens)
                bias_sb = bias_kill_sb if t <= LAG else bias_main_sb
                gp = {}
                for gi in GATE_ORDER:
                    gp[gi] = pg.tile([BS, H], F32, tag="g", name=f"g{gi}_{t}")
                if not SKIP_MM:
                    for pair in ((0, 2), (1, 3)):
                        for k in range(KC):
                            st = sel_cur[:, k * BS:(k + 1) * BS]
                            for gi in pair:
                                for n in range(2):
                                    nc.tensor.matmul(
                                        gp[gi][:, n * 512:(n + 1) * 512], st,
                                        w_in_sb[:, k * G4 + gi * H + n * 512:
                                                k * G4 + gi * H + (n + 1) * 512],
                                        start=(k == 0), stop=False)
                else:
                    for gi in GATE_ORDER:
                        nc.vector.memset(gp[gi][:], 0.0)

                # ---------- transpose h(t-1) -> hT slab (PE), ship h0T(t-1)
                if t >= 1:
                    tr = ptr.tile([128, SLAB], F32, tag="tr", name=f"tr{t}")
                    for k in range(KC):
                        nc.tensor.transpose(tr[:, k * BS:(k + 1) * BS],
                                            h_sb[:, k * 128:(k + 1) * 128],
                                            id_sb[:])
                    hT_prev = hpool.tile([128, SLAB], BF16, tag="hT",
                                         name=f"hT{t - 1}")
                    nc.scalar.activation(hT_prev[:], tr[:], AF.Copy)
                    yi = t - 1 - LAG - 1
                    if yi >= 0:
                        nc.scalar.dma_start(y[yi], hT_prev[:])
                    if t - 1 < t_steps and not SKIP_CC:
                        cc_in = dpool.tile([128, SLAB], BF16, tag="ci",
                                           name=f"ci{t - 1}")
                        cc_out = dpool.tile([2, 128, SLAB], BF16,
                                            tag="co", name=f"co{t - 1}")
                        nc.scalar.dma_start(cc_in[:], hT_prev[:])
                        nc.gpsimd.collective_compute(
                            "AllGather", mybir.AluOpType.bypass,
                            ins=[cc_in.opt()], outs=[cc_out.opt()],
                            replica_groups=rg,
                        )
                        cc_outs[t - 1] = cc_out

                # ---------- gates(t): rec-part (psum closes)
                if not SKIP_MM:
                    for pair in ((0, 2), (1, 3)):
                        for k in range(KC):
                            st = hT_prev[:, k * BS:(k + 1) * BS]
                            for gi in pair:
                                for n in range(2):
                                    nc.tensor.matmul(
                                        gp[gi][:, n * 512:(n + 1) * 512], st,
                                        w_rec_sb[:, k * G4 + gi * H + n * 512:
                                                 k * G4 + gi * H + (n + 1) * 512],
                                        start=False,
                                        stop=(k == KC - 1))

                # ---------- sel(t+1) ahead of the cell's DVE work
                if t + 1 < S:
                    sel_next = build_sel(t + 1)

                # ---------- cell: bias add (DVE) + activations (ACT) + state
                if not SKIP_CELL:
                    pre = {}
                    for gi in (0, 2, 1, 3):
                        pre[gi] = apool.tile([BS, H], F32, tag=f"p{gi}",
                                             name=f"p{gi}_{t}")
                        nc.vector.tensor_add(
                            pre[gi][:], gp[gi][:],
                            bias_sb[:, gi * H:(gi + 1) * H])
                    a_i = apool.tile([BS, H], F32, tag="ai")
                    a_g = apool.tile([BS, H], F32, tag="ag_")
                    a_f = apool.tile([BS, H], F32, tag="af")
                    a_o = apool.tile([BS, H], F32, tag="ao")
                    nc.scalar.activation(a_i[:], pre[0][:], AF.Sigmoid)
                    nc.scalar.activation(a_g[:], pre[2][:], AF.Tanh)
                    nc.scalar.activation(a_f[:], pre[1][:], AF.Sigmoid)
                    nc.scalar.activation(a_o[:], pre[3][:], AF.Sigmoid)
                    ig = apool.tile([BS, H], F32, tag="ig")
                    nc.vector.tensor_mul(ig[:], a_i[:], a_g[:])
                    nc.vector.tensor_mul(c_st[:], c_st[:], a_f[:])
                    nc.vector.tensor_add(c_st[:], c_st[:], ig[:])
                    th = apool.tile([BS, H], F32, tag="th")
                    nc.scalar.activation(th[:], c_st[:], AF.Tanh)
                    h_sb = apool.tile([BS, H], F32, tag="h")
                    nc.vector.tensor_mul(h_sb[:], a_o[:], th[:])
                else:
                    h_sb = apool.tile([BS, H], F32, tag="h")
                    nc.scalar.activation(h_sb[:], gp[3][:], AF.Sigmoid)

                sel_cur = sel_next

            # ---------- epilogue: flush h(S-1) -> y[t_steps-1]
            tr = ptr.tile([128, SLAB], F32, tag="tr", name="tr_end")
            for k in range(KC):
                nc.tensor.transpose(tr[:, k * BS:(k + 1) * BS],
                                    h_sb[:, k * 128:(k + 1) * 128], id_sb[:])
            hT_last = hpool.tile([128, SLAB], BF16, tag="hT", name="hT_end")
            nc.scalar.activation(hT_last[:], tr[:], AF.Copy)
            nc.scalar.dma_start(y[t_steps - 1], hT_last[:])

    _dedup_ldweights(nc)
    _hoist_multi_waits(nc)
    return nc


def _prep_inputs(x, W_ih0, b_ih0, W_hh0, b_hh0, W_ih1, b_ih1, W_hh1, b_hh1,
                 t_steps: int):
    """Per-core tensors; role (layer 0/1) and pair batch slice from core id."""
    bf = ml_dtypes.bfloat16
    S = t_steps + LAG + 1

    def wmov(W):
        # [4096, 1024] -> moving layout [128, KC * 4096]
        Wt = np.ascontiguousarray(W.T).reshape(KC, 128, G4).transpose(1, 0, 2)
        return np.ascontiguousarray(Wt).reshape(128, KC * G4).astype(bf)

    w_in0, w_rec0 = wmov(W_ih0), wmov(W_hh0)
    w_in1, w_rec1 = wmov(W_ih1), wmov(W_hh1)
    bias0 = np.broadcast_to((b_ih0 + b_hh0)[None, :], (BS, G4)).astype(bf)
    bias1 = np.broadcast_to((b_ih1 + b_hh1)[None, :], (BS, G4)).astype(bf)
    bias_zero = np.zeros((BS, G4), dtype=bf)
    ident = np.eye(BS, dtype=np.float32)

    zero_x = np.zeros((S, 128, SLAB), dtype=bf)
    mask0 = np.zeros((128, SLAB), dtype=np.uint8)
    mask1 = np.ones((128, SLAB), dtype=np.uint8)

    in_maps = []
    for g in range(N_PAIRS):
        xs = x[g * BS:(g + 1) * BS, :t_steps, :]            # [32, T, 1024]
        xt = np.ascontiguousarray(xs.transpose(1, 2, 0))    # [T, 1024, 32]
        xt = xt.reshape(t_steps, KC, 128, BS).transpose(0, 2, 1, 3)
        xt = np.ascontiguousarray(xt).reshape(t_steps, 128, SLAB).astype(bf)
        xA = np.concatenate(
            [xt, np.zeros((S - t_steps, 128, SLAB), dtype=bf)], axis=0)

        in_maps.append({  # even core: layer 0
            "xT": xA, "w_in": w_in0, "w_rec": w_rec0,
            "bias_main": bias0, "bias_kill": bias0,
            "masks": mask0, "ident": ident,
        })
        in_maps.append({  # odd core: layer 1
            "xT": zero_x, "w_in": w_in1, "w_rec": w_rec1,
            "bias_main": bias1, "bias_kill": bias_zero,
            "masks": mask1, "ident": ident,
        })
    return in_maps


_NC_CACHE: dict[int, bass.Bass] = {}


def run_pair_lstm(inputs: dict, t_steps: int = T_FULL, trace: bool = False):
    in_maps = _prep_inputs(**inputs, t_steps=t_steps)
    if t_steps not in _NC_CACHE:
        _NC_CACHE[t_steps] = build_pair_nc(t_steps)
    nc = _NC_CACHE[t_steps]
    res = run_bass_kernel_spmd(nc, in_maps, list(range(N_CORES)), trace=trace)
    # y arrives as transposed bf16 slabs [T, 128, KC*BS]; de-transpose on host:
    # y[32g+j, t, k*128+p] = yT[t, p, k*32+j]
    parts = []
    for g in range(N_PAIRS):
        yT = np.asarray(res.results[2 * g + 1]["y"])        # [T, 128, 256]
        yT = yT.reshape(t_steps, 128, KC, BS).transpose(3, 0, 2, 1)
        parts.append(np.ascontiguousarray(yT).reshape(BS, t_steps, H)
                     .astype(np.float32))
    y = np.concatenate(parts, axis=0)
    return y, res


def kernel(**inputs) -> np.ndarray:
    y, _ = run_pair_lstm(inputs, t_steps=T_FULL, trace=False)
    return y


if __name__ == "__main__":
    rng = np.random.default_rng(0)
    sc = 1.0 / np.sqrt(F)
    ins = {
        "x": rng.standard_normal((B, T_FULL, F)).astype(np.float32),
        "W_ih0": (rng.standard_normal((4 * H, F)) * sc).astype(np.float32),
        "b_ih0": (rng.standard_normal(4 * H) * sc).astype(np.float32),
        "W_hh0": (rng.standard_normal((4 * H, H)) * sc).astype(np.float32),
        "b_hh0": (rng.standard_normal(4 * H) * sc).astype(np.float32),
        "W_ih1": (rng.standard_normal((4 * H, H)) * sc).astype(np.float32),
        "b_ih1": (rng.standard_normal(4 * H) * sc).astype(np.float32),
        "W_hh1": (rng.standard_normal((4 * H, H)) * sc).astype(np.float32),
        "b_hh1": (rng.standard_normal(4 * H) * sc).astype(np.float32),
    }
    y, res = run_pair_lstm(ins, t_steps=16)
    print("y shape", y.shape)


# revision 18
# speedup vs baseline: 6.6191x; 6.6191x over previous
"""2-layer LSTM (B=128, T=256, F=H=1024) on 8 Trainium2 NeuronCores.

Pair-pipeline: 4 pairs of cores; pair g owns batch slice [32g, 32g+32).
The even core of each pair runs layer 0, the odd core layer 1 — the SAME
SPMD program, role encoded purely in per-core input data (weights, mask,
bias-kill sequence). Each layer's recurrence is core-local; h0 streams
even->odd through one pairwise AllGather per step, consumed with a lag of
LAG steps, so no collective sits on the serial critical path.

Per core, per step t the PE queue is ordered to hide the ACT/DVE cell
chain of step t-1 under work that does not depend on it:

    [bias mm(t) + in-part mm(t)]  [transpose h(t-1)]  [rec-part mm(t)]

gates(t) psum accumulates bias (ones-row matmul) + sel(t) @ W_in first
(sel = x for even cores, gathered h0 for odd, chosen by one
copy_predicated), and closes with hT(t-1) @ W_rec. The cell (sigmoid/tanh
+ DVE state update) runs on ACT/DVE while the PE chews the next step's
in-part, so the step period ~= PE busy time (~29.5 us).
"""

import numpy as np
import ml_dtypes

import concourse.bass as bass
import concourse.tile as tile
from concourse import mybir

SKIP_CC = SKIP_MM = SKIP_CELL = False
N_CORES = 8
N_PAIRS = 4
BS = 32            # batch rows per pair
B = 128
T_FULL = 256
F = 1024
H = 1024
G4 = 4 * H         # 4096 gate columns
KC = 8             # 128-row K chunks for a 1024 contraction
LAG = 4            # AllGather consume lag (steps)
PF = 3             # DMA prefetch depth (steps)
BF16 = mybir.dt.bfloat16
F32 = mybir.dt.float32
AF = mybir.ActivationFunctionType
SLAB = KC * BS     # 256: hT / x slab free width


def _patch_tail_drain():
    """walrus on this image only allows ONE sem-wait on CTRL-type (Drain/NoOp)
    instructions; Tile's kernel-tail drain accumulates one wait per pending
    queue/collective sem and trips that limit. Spread the waits over a chain
    of single-wait nops instead."""
    if getattr(tile.TileContext, "_tail_drain_patched", False):
        return
    from concourse.tile import ScopedClock

    def _drain_and_barrier(self, tick_clock, wait_clock):
        nc = self.nc
        probe = nc.sync.nop(nofuse=True, hint="tail_wait_probe")
        wait_clock.add_sem_waits(probe.ins, ScopedClock({None: tick_clock.global_clock}))
        si = probe.ins.sync_info
        waits = list(si.on_wait) if si is not None else []
        if len(waits) > 1:
            si.on_wait = waits[:1]
            for w in waits[1:]:
                n2 = nc.sync.nop(nofuse=True, hint="tail_wait_extra")
                n2.ins.sync_info = mybir.SyncInfo(on_wait=[w], on_update=[])
        nc.sync.drain()
        nc.all_engine_barrier()
        popped = nc._tile_sem_poison_stack.pop()
        assert popped is self._sem_poison
        nc.clear_and_free_semaphores(list(self.sems.allocated().values()))
        nc.all_engine_barrier()

    tile.TileContext._drain_and_barrier = _drain_and_barrier
    tile.TileContext._tail_drain_patched = True


def _hoist_multi_waits(nc: bass.Bass):
    """walrus on this image rejects >1 sem-wait per instruction. Hoist extra
    waits onto single-wait NoOps inserted just before the instruction on the
    same engine (engine FIFO order makes this equivalent)."""
    for blk in nc.main_func.blocks:
        idx = 0
        while idx < len(blk.instructions):
            inst = blk.instructions[idx]
            si = getattr(inst, "sync_info", None)
            if si is not None and len(si.on_wait) > 1:
                waits = list(si.on_wait)
                si.on_wait = waits[-1:]
                for w in waits[:-1]:
                    nop = mybir.InstNoOp(
                        name=nc.get_next_instruction_name(), ins=[], outs=[]
                    )
                    nop.engine = inst.engine
                    nop.sync_info = mybir.SyncInfo(on_wait=[w], on_update=[])
                    nc.register_instruction(nop)
                    blk.instructions.insert(idx, nop)
                    idx += 1
            idx += 1


def _ap_sig(arg):
    """Stable signature for a lowered AP argument (stationary operand)."""
    try:
        ml = arg.memory_location()
        name = ml.name if ml is not None else None
    except Exception:
        name = getattr(arg, "name", None)
    return (name, getattr(arg, "offset", None), str(getattr(arg, "ap", None)),
            str(getattr(arg, "dtype", None)))


def _dedup_ldweights(nc: bass.Bass):
    """Drop InstLdweights that reload the stationary already in the PE array.

    The PE keeps the loaded stationary across matmuls; only another
    InstLdweights or a self-loading matmul (transpose) changes it. Deleted
    instructions' sem waits/updates are merged onto the next PE instruction
    (engine FIFO order preserves timing semantics); _hoist_multi_waits runs
    afterwards to restore the single-wait invariant.
    """
    for blk in nc.main_func.blocks:
        cur_sig = None
        new_instructions = []
        pending_sync = []
        for inst in blk.instructions:
            eng = getattr(inst, "engine", None)
            if eng == mybir.EngineType.PE:
                if isinstance(inst, mybir.InstLdweights):
                    sig = _ap_sig(inst.ins[0])
                    if sig == cur_sig:
                        si = getattr(inst, "sync_info", None)
                        if si is not None and (si.on_wait or si.on_update):
                            pending_sync.append(si)
                        continue        # drop redundant reload
                    cur_sig = sig
                elif isinstance(inst, mybir.InstMatmult):
                    if getattr(inst, "is_transpose", False):
                        cur_sig = None  # transpose self-loads the identity
                if pending_sync:
                    si = getattr(inst, "sync_info", None)
                    if si is None:
                        si = mybir.SyncInfo(on_wait=[], on_update=[])
                        inst.sync_info = si
                    for ps in pending_sync:
                        si.on_wait = list(si.on_wait) + list(ps.on_wait)
                        si.on_update = list(si.on_update) + list(ps.on_update)
                    pending_sync = []
            new_instructions.append(inst)
        assert not pending_sync, "dangling sync from trailing Ldweights"
        blk.instructions[:] = new_instructions


def build_pair_nc(t_steps: int) -> bass.Bass:
    _patch_tail_drain()
    nc = bass.Bass()
    S = t_steps + LAG + 1

    xT = nc.dram_tensor("xT", [S, 128, SLAB], BF16, kind="ExternalInput")
    w_in = nc.dram_tensor("w_in", [128, KC * G4], BF16, kind="ExternalInput")
    w_rec = nc.dram_tensor("w_rec", [128, KC * G4], BF16, kind="ExternalInput")
    # batch-replicated bias rows; _kill is the variant used for t <= LAG
    # (zeros on odd cores so layer-1 state stays exactly 0 during pipe fill)
    bias_main = nc.dram_tensor("bias_main", [BS, G4], BF16,
                               kind="ExternalInput")
    bias_kill = nc.dram_tensor("bias_kill", [BS, G4], BF16,
                               kind="ExternalInput")
    masks = nc.dram_tensor("masks", [128, SLAB], mybir.dt.uint8,
                           kind="ExternalInput")
    ident = nc.dram_tensor("ident", [BS, BS], F32, kind="ExternalInput")
    y = nc.dram_tensor("y", [t_steps, 128, SLAB], BF16, kind="ExternalOutput")

    # pairwise replica groups: pair g = cores (2g, 2g+1)
    rg = [[2 * g, 2 * g + 1] for g in range(N_PAIRS)]

    with tile.TileContext(nc) as tc:
        with (
            tc.tile_pool(name="const", bufs=1) as cpool,
            tc.tile_pool(name="xbuf", bufs=PF + 2) as xpool,
            tc.tile_pool(name="agbuf", bufs=PF + 2) as agpool,
            tc.tile_pool(name="selbuf", bufs=3) as selpool,
            tc.tile_pool(name="hslab", bufs=3) as hpool,
            tc.tile_pool(name="state", bufs=1) as spool,
            tc.tile_pool(name="act", bufs=1) as apool,
            tc.tile_pool(name="pg", bufs=3, space="PSUM") as pg,
            tc.tile_pool(name="ptr", bufs=2, space="PSUM") as ptr,
            tc.tile_pool(name="dram", bufs=LAG + 4, space="DRAM") as dpool,
        ):
            # ---- constants resident in SBUF (weights loaded per K-chunk so
            # the first matmuls start after 1/8 of the load)
            w_in_sb = cpool.tile([128, KC * G4], BF16)
            w_rec_sb = cpool.tile([128, KC * G4], BF16)
            for k in range(KC):
                nc.sync.dma_start(w_in_sb[:, k * G4:(k + 1) * G4],
                                  w_in[:, k * G4:(k + 1) * G4])
            for k in range(KC):
                nc.sync.dma_start(w_rec_sb[:, k * G4:(k + 1) * G4],
                                  w_rec[:, k * G4:(k + 1) * G4])
            bias_main_sb = cpool.tile([BS, G4], BF16)
            bias_kill_sb = cpool.tile([BS, G4], BF16)
            masks_sb = cpool.tile([128, SLAB], mybir.dt.uint8)
            id_sb = cpool.tile([BS, BS], F32)
            nc.sync.dma_start(bias_main_sb[:], bias_main[:])
            nc.sync.dma_start(bias_kill_sb[:], bias_kill[:])
            nc.sync.dma_start(masks_sb[:], masks[:])
            nc.sync.dma_start(id_sb[:], ident[:])

            # ---- persistent state
            c_st = spool.tile([BS, H], F32)
            nc.vector.memset(c_st[:], 0.0)
            hT_init = cpool.tile([128, SLAB], BF16)
            nc.vector.memset(hT_init[:], 0.0)

            # ---- prefetched x / ag slabs
            x_slabs, ag_reads = {}, {}
            for t in range(min(PF, S)):
                xs = xpool.tile([128, SLAB], BF16, tag="x", name=f"xpre{t}")
                nc.sync.dma_start(xs[:], xT[t])
                x_slabs[t] = xs

            cc_outs = {}    # production step -> shared DRAM tile

            def build_sel(t):
                """sel(t) = x(t), overwritten with gathered h0T where mask=1."""
                sel = selpool.tile([128, SLAB], BF16, tag="sel", name=f"sel{t}")
                nc.vector.tensor_copy(sel[:], x_slabs.pop(t)[:])
                if t in ag_reads:
                    nc.vector.copy_predicated(sel[:], masks_sb[:],
                                              ag_reads.pop(t)[:])
                return sel

            sel_cur = build_sel(0)
            sel_next = None
            hT_prev = hT_init
            GATE_ORDER = (0, 2, 1, 3)  # i, g, f, o (W col blocks: i|f|g|o)

            for t in range(S):
                # ---------- prefetch DMAs for step t+PF
                tp = t + PF
                if tp < S:
                    xs = xpool.tile([128, SLAB], BF16, tag="x", name=f"x{tp}")
                    nc.sync.dma_start(xs[:], xT[tp])
                    x_slabs[tp] = xs
                    src = tp - LAG - 1
                    if src in cc_outs:
                        ag = agpool.tile([128, SLAB], BF16, tag="ag",
                                         name=f"ag{tp}")
                        nc.sync.dma_start(ag[:], cc_outs.pop(src)[0])
                        ag_reads[tp] = ag

                # ---------- gates(t): in-part (psum opens)
                bias_sb = bias_kill_sb if t <= LAG else bias_main_sb
                gp = {}
                for gi in GATE_ORDER:
                    gp[gi] = pg.tile([BS, H], F32, tag="g", name=f"g{gi}_{t}")
                if not SKIP_MM:
                    for pair in ((0, 2), (1, 3)):
                        for k in range(KC):
                            st = sel_cur[:, k * BS:(k + 1) * BS]
                            for gi in pair:
                                for n in range(2):
                                    nc.tensor.matmul(
                                        gp[gi][:, n * 512:(n + 1) * 512], st,
                                        w_in_sb[:, k * G4 + gi * H + n * 512:
                                                k * G4 + gi * H + (n + 1) * 512],
                                        start=(k == 0), stop=False)
                else:
                    for gi in GATE_ORDER:
                        nc.vector.memset(gp[gi][:], 0.0)

                # ---------- transpose h(t-1) -> hT slab (PE), ship h0T(t-1)
                if t >= 1:
                    tr = ptr.tile([128, SLAB], F32, tag="tr", name=f"tr{t}")
                    for k in range(KC):
                        nc.tensor.transpose(tr[:, k * BS:(k + 1) * BS],
                                            h_sb[:, k * 128:(k + 1) * 128],
                                            id_sb[:])
                    hT_prev = hpool.tile([128, SLAB], BF16, tag="hT",
                                         name=f"hT{t - 1}")
                    nc.scalar.activation(hT_prev[:], tr[:], AF.Copy)
                    yi = t - 1 - LAG - 1
                    if yi >= 0:
                        nc.scalar.dma_start(y[yi], hT_prev[:])
                    if t - 1 < t_steps and not SKIP_CC:
                        cc_in = dpool.tile([128, SLAB], BF16, tag="ci",
                                           name=f"ci{t - 1}")
                        cc_out = dpool.tile([2, 128, SLAB], BF16,
                                            tag="co", name=f"co{t - 1}")
                        nc.scalar.dma_start(cc_in[:], hT_prev[:])
                        nc.gpsimd.collective_compute(
                            "AllGather", mybir.AluOpType.bypass,
                            ins=[cc_in.opt()], outs=[cc_out.opt()],
                            replica_groups=rg,
                        )
                        cc_outs[t - 1] = cc_out

                # ---------- gates(t): rec-part (psum closes)
                if not SKIP_MM:
                    for pair in ((0, 2), (1, 3)):
                        for k in range(KC):
                            st = hT_prev[:, k * BS:(k + 1) * BS]
                            for gi in pair:
                                for n in range(2):
                                    nc.tensor.matmul(
                                        gp[gi][:, n * 512:(n + 1) * 512], st,
                                        w_rec_sb[:, k * G4 + gi * H + n * 512:
                                                 k * G4 + gi * H + (n + 1) * 512],
                                        start=False,
                                        stop=(k == KC - 1))

                # ---------- sel(t+1) ahead of the cell's DVE work
                if t + 1 < S:
                    sel_next = build_sel(t + 1)

                # ---------- cell: bias add (DVE) + activations (ACT) + state
                if not SKIP_CELL:
                    pre = {}
                    for gi in (0, 2, 1, 3):
                        pre[gi] = apool.tile([BS, H], F32, tag=f"p{gi}",
                                             name=f"p{gi}_{t}")
                        nc.vector.tensor_add(
                            pre[gi][:], gp[gi][:],
                            bias_sb[:, gi * H:(gi + 1) * H])
                    a_i = apool.tile([BS, H], F32, tag="ai")
                    a_g = apool.tile([BS, H], F32, tag="ag_")
                    a_f = apool.tile([BS, H], F32, tag="af")
                    a_o = apool.tile([BS, H], F32, tag="ao")
                    nc.scalar.activation(a_i[:], pre[0][:], AF.Sigmoid)
                    nc.scalar.activation(a_g[:], pre[2][:], AF.Tanh)
                    nc.scalar.activation(a_f[:], pre[1][:], AF.Sigmoid)
                    nc.scalar.activation(a_o[:], pre[3][:], AF.Sigmoid)
                    ig = apool.tile([BS, H], F32, tag="ig")
                    nc.vector.tensor_mul(ig[:], a_i[:], a_g[:])
                    nc.vector.tensor_mul(c_st[:], c_st[:], a_f[:])
                    nc.vector.tensor_add(c_st[:], c_st[:], ig[:])
                    th = apool.tile([BS, H], F32, tag="th")
                    nc.scalar.activation(th[:], c_st[:], AF.Tanh)
                    h_sb = apool.tile([BS, H], F32, tag="h")
                    nc.vector.tensor_mul(h_sb[:], a_o[:], th[:])
                else:
                    h_sb = apool.tile([BS, H], F32, tag="h")
                    nc.scalar.activation(h_sb[:], gp[3][:], AF.Sigmoid)

                sel_cur = sel_next

            # ---------- epilogue: flush h(S-1) -> y[t_steps-1]
            tr = ptr.tile([128, SLAB], F32, tag="tr", name="tr_end")
            for k in range(KC):
                nc.tensor.transpose(tr[:, k * BS:(k + 1) * BS],
                                    h_sb[:, k * 128:(k + 1) * 128], id_sb[:])
            hT_last = hpool.tile([128, SLAB], BF16, tag="hT", name="hT_end")
            nc.scalar.activation(hT_last[:], tr[:], AF.Copy)
            nc.scalar.dma_start(y[t_steps - 1], hT_last[:])

    _dedup_ldweights(nc)
    _hoist_multi_waits(nc)
    return nc


def _prep_inputs(x, W_ih0, b_ih0, W_hh0, b_hh0, W_ih1, b_ih1, W_hh1, b_hh1,
                 t_steps: int):
    """Per-core tensors; role (layer 0/1) and pair batch slice from core id."""
    bf = ml_dtypes.bfloat16
    S = t_steps + LAG + 1

    def wmov(W):
        # [4096, 1024] -> moving layout [128, KC * 4096]
        Wt = np.ascontiguousarray(W.T).reshape(KC, 128, G4).transpose(1, 0, 2)
        return np.ascontiguousarray(Wt).reshape(128, KC * G4).astype(bf)

    w_in0, w_rec0 = wmov(W_ih0), wmov(W_hh0)
    w_in1, w_rec1 = wmov(W_ih1), wmov(W_hh1)
    bias0 = np.broadcast_to((b_ih0 + b_hh0)[None, :], (BS, G4)).astype(bf)
    bias1 = np.broadcast_to((b_ih1 + b_hh1)[None, :], (BS, G4)).astype(bf)
    bias_zero = np.zeros((BS, G4), dtype=bf)
    ident = np.eye(BS, dtype=np.float32)

    zero_x = np.zeros((S, 128, SLAB), dtype=bf)
    mask0 = np.zeros((128, SLAB), dtype=np.uint8)
    mask1 = np.ones((128, SLAB), dtype=np.uint8)

    in_maps = []
    for g in range(N_PAIRS):
        xs = x[g * BS:(g + 1) * BS, :t_steps, :]            # [32, T, 1024]
        xt = np.ascontiguousarray(xs.transpose(1, 2, 0))    # [T, 1024, 32]
        xt = xt.reshape(t_steps, KC, 128, BS).transpose(0, 2, 1, 3)
        xt = np.ascontiguousarray(xt).reshape(t_steps, 128, SLAB).astype(bf)
        xA = np.concatenate(
            [xt, np.zeros((S - t_steps, 128, SLAB), dtype=bf)], axis=0)

        in_maps.append({  # even core: layer 0
            "xT": xA, "w_in": w_in0, "w_rec": w_rec0,
            "bias_main": bias0, "bias_kill": bias0,
            "masks": mask0, "ident": ident,
        })
        in_maps.append({  # odd core: layer 1
            "xT": zero_x, "w_in": w_in1, "w_rec": w_rec1,
            "bias_main": bias1, "bias_kill": bias_zero,
            "masks": mask1, "ident": ident,
        })
    return in_maps


_NC_CACHE: dict[int, bass.Bass] = {}
_RUNNER_CACHE: dict[int, object] = {}


def _make_runner(nc: bass.Bass, n_cores: int):
    """Cached PJRT executable for repeat kernel() calls (run_bass_via_pjrt
    rebuilds its jit closure every call, costing a full XLA retrace)."""
    import jax
    from jax.experimental.shard_map import shard_map
    from jax.sharding import Mesh, PartitionSpec
    from concourse import bass2jax

    bass2jax.install_neuronx_cc_hook()
    partition_name = (nc.partition_id_tensor.name
                      if nc.partition_id_tensor else None)
    in_names, out_names, out_avals, zero_outs = [], [], [], []
    for alloc in nc.m.functions[0].allocations:
        if not isinstance(alloc, mybir.MemoryLocationSet):
            continue
        name = alloc.memorylocations[0].name
        if alloc.kind == "ExternalInput":
            if name != partition_name:
                in_names.append(name)
        elif alloc.kind == "ExternalOutput":
            out_names.append(name)
            shape = tuple(alloc.tensor_shape)
            dtype = mybir.dt.np(alloc.dtype)
            out_avals.append(jax.core.ShapedArray(shape, dtype))
            zero_outs.append(np.zeros(shape, dtype))
    n_params = len(in_names)
    n_outs = len(out_avals)
    all_in_names = list(in_names) + list(out_names)
    if partition_name is not None:
        all_in_names.append(partition_name)
    donate = tuple(range(n_params, n_params + n_outs))

    def _body(*args):
        operands = list(args)
        if partition_name is not None:
            operands.append(bass2jax.partition_id_tensor())
        outs = bass2jax._bass_exec_p.bind(
            *operands, out_avals=tuple(out_avals),
            in_names=tuple(all_in_names), out_names=tuple(out_names),
            lowering_input_output_aliases=(),
            sim_require_finite=True, sim_require_nnan=True, nc=nc,
        )
        return tuple(outs)

    devices = jax.devices()[:n_cores]
    mesh = Mesh(np.asarray(devices), ("core",))
    specs = (PartitionSpec("core"),)
    sharded = jax.jit(
        shard_map(_body, mesh=mesh, in_specs=specs * (n_params + n_outs),
                  out_specs=specs * len(out_names), check_rep=False),
        donate_argnums=donate, keep_unused=True,
    )

    def runner(in_maps):
        concat_in = [
            np.concatenate([np.asarray(m[name]) for m in in_maps], axis=0)
            for name in in_names
        ]
        zeros = [np.zeros((n_cores * z.shape[0], *z.shape[1:]), z.dtype)
                 for z in zero_outs]
        out_arrs = sharded(*concat_in, *zeros)
        return [
            {name: np.asarray(out_arrs[i]).reshape(n_cores,
                                                   *out_avals[i].shape)[c]
             for i, name in enumerate(out_names)}
            for c in range(n_cores)
        ]

    return runner


def run_pair_lstm(inputs: dict, t_steps: int = T_FULL, trace: bool = False):
    in_maps = _prep_inputs(**inputs, t_steps=t_steps)
    if t_steps not in _NC_CACHE:
        _NC_CACHE[t_steps] = build_pair_nc(t_steps)
    nc = _NC_CACHE[t_steps]
    if t_steps not in _RUNNER_CACHE:
        _RUNNER_CACHE[t_steps] = _make_runner(nc, N_CORES)
    results = _RUNNER_CACHE[t_steps](in_maps)
    # y arrives as transposed bf16 slabs [T, 128, KC*BS]; de-transpose on host:
    # y[32g+j, t, k*128+p] = yT[t, p, k*32+j]
    parts = []
    for g in range(N_PAIRS):
        yT = np.asarray(results[2 * g + 1]["y"])            # [T, 128, 256]
        yT = yT.reshape(t_steps, 128, KC, BS).transpose(3, 0, 2, 1)
        parts.append(np.ascontiguousarray(yT).reshape(BS, t_steps, H)
                     .astype(np.float32))
    y = np.concatenate(parts, axis=0)
    return y, results


def kernel(**inputs) -> np.ndarray:
    y, _ = run_pair_lstm(inputs, t_steps=T_FULL)
    return y


if __name__ == "__main__":
    rng = np.random.default_rng(0)
    sc = 1.0 / np.sqrt(F)
    ins = {
        "x": rng.standard_normal((B, T_FULL, F)).astype(np.float32),
        "W_ih0": (rng.standard_normal((4 * H, F)) * sc).astype(np.float32),
        "b_ih0": (rng.standard_normal(4 * H) * sc).astype(np.float32),
        "W_hh0": (rng.standard_normal((4 * H, H)) * sc).astype(np.float32),
        "b_hh0": (rng.standard_normal(4 * H) * sc).astype(np.float32),
        "W_ih1": (rng.standard_normal((4 * H, H)) * sc).astype(np.float32),
        "b_ih1": (rng.standard_normal(4 * H) * sc).astype(np.float32),
        "W_hh1": (rng.standard_normal((4 * H, H)) * sc).astype(np.float32),
        "b_hh1": (rng.standard_normal(4 * H) * sc).astype(np.float32),
    }
    y, res = run_pair_lstm(ins, t_steps=16)
    print("y shape", y.shape)
